# revision 15
# baseline (speedup 1.0000x reference)
"""Trainium2 Bass kernel for nn_DANLayer (dense per-neuron MLP with BatchNorm).

Strategy
--------
Shard the neuron axis N=64 across 8 cores (8 neurons/core). Each core sees the
full batch, so BatchNorm batch statistics need no collectives.

Math folding (validated against the jax reference to ~1e-6 in fp32):
  BN1 affine + normalization fold into lin1:
      h1 = tanh(x @ (W1 * (rs1*g1)[:,None]) + (b1 + (beta1 - mu1*rs1*g1) @ W1))
  BN2 likewise folds into lin2. The kernel never materializes xhat/h1h —
  x is the raw matmul moving operand; all per-feature scales live in weights.

Layout: x loads batch-major, is PE-transposed (through PSUM) and stored
feature-major in SBUF as bf16. h1/h2 are feature-major bf16. Matmuls run in
bf16 (1 cycle/row on TRN2 vs 4 for fp32) with fp32 PSUM accumulation. lin3
streams W3 as split-bf16 (hi+lo accumulated in PSUM) to keep the output
matmul near-fp32. rsqrt is computed with the int-bit-trick + 3 Newton steps
on GPSIMD to keep the ACT table pinned to the tanh set (a Sqrt would force a
~1.3us table reload per neuron). Outputs are staged per neuron-pair so DMA
rows are 512B contiguous.
"""

import sys

for _p in ("/opt/trn_rl_repo", "/root/.axon_site/_ro/trn_rl_repo"):
    if _p not in sys.path:
        sys.path.insert(0, _p)

from contextlib import ExitStack

import numpy as np

import concourse.bass as bass
import concourse.tile as tile
from concourse import bacc, mybir
from concourse.bass_utils import run_bass_kernel_spmd
from concourse.masks import make_identity

B, N, NI, NF, NO = 8192, 64, 256, 128, 64
NCORES = 8
NPC = N // NCORES            # neurons per core
EPS = 1e-5
KC = NI // 128               # lin1 contraction chunks

FP32 = mybir.dt.float32
BF16 = mybir.dt.bfloat16
I32 = mybir.dt.int32

AL = mybir.AluOpType
AF = mybir.ActivationFunctionType

NBT = B // 128               # 64 batch tiles
CHUNK = 2048                 # psum chunk (4 banks)
NCH = B // CHUNK             # 4 chunks per full-batch sweep
MMN = 512                    # matmul free dim (one PSUM bank)
RSQRT_MAGIC = 0x5F3759DF


def build_program():
    nc = bacc.Bacc("TRN2", target_bir_lowering=False, debug=False)

    x_d = nc.dram_tensor("x", [B, NI], FP32, kind="ExternalInput").ap()
    w1_d = nc.dram_tensor("W1s", [NPC, NI, NF], FP32, kind="ExternalInput").ap()
    b1_d = nc.dram_tensor("b1s", [NPC, NF], FP32, kind="ExternalInput").ap()
    w2_d = nc.dram_tensor("W2s", [NPC, NF, NF], FP32, kind="ExternalInput").ap()
    b2_d = nc.dram_tensor("b2s", [NPC, NF], FP32, kind="ExternalInput").ap()
    w3_d = nc.dram_tensor("W3s", [NPC, NF, NO], FP32, kind="ExternalInput").ap()
    b3_d = nc.dram_tensor("b3s", [NPC, NO], FP32, kind="ExternalInput").ap()
    g1_d = nc.dram_tensor("g1s", [NPC, NI], FP32, kind="ExternalInput").ap()
    be1_d = nc.dram_tensor("beta1s", [NPC, NI], FP32, kind="ExternalInput").ap()
    g2_d = nc.dram_tensor("g2s", [NPC, NF], FP32, kind="ExternalInput").ap()
    be2_d = nc.dram_tensor("beta2s", [NPC, NF], FP32, kind="ExternalInput").ap()
    out_d = nc.dram_tensor("out", [B, NPC * NO], FP32, kind="ExternalOutput").ap()

    with tile.TileContext(nc) as tc:
        with ExitStack() as ctx:
            kernel_body(ctx, tc, x_d, w1_d, b1_d, w2_d, b2_d, w3_d, b3_d,
                        g1_d, be1_d, g2_d, be2_d, out_d)
    nc.compile()
    return nc


def kernel_body(ctx, tc, x_d, w1_d, b1_d, w2_d, b2_d, w3_d, b3_d,
                g1_d, be1_d, g2_d, be2_d, out_d):
    nc = tc.nc

    const = ctx.enter_context(tc.tile_pool(name="const", bufs=1))
    xload = ctx.enter_context(tc.tile_pool(name="xload", bufs=4))
    hpool = ctx.enter_context(tc.tile_pool(name="hpool", bufs=2))
    stg_pool = ctx.enter_context(tc.tile_pool(name="stgp", bufs=2))
    vec = ctx.enter_context(tc.tile_pool(name="vec", bufs=2))
    wef = ctx.enter_context(tc.tile_pool(name="wef", bufs=2))
    mm_ps = ctx.enter_context(tc.tile_pool(name="mm_ps", bufs=2, space="PSUM"))

    def ctile(shape, dtype, tag):
        return const.tile(shape, dtype, tag=tag, name=tag)

    # ---------------- constants / weight loads ----------------
    identity = ctile([128, 128], FP32, "identity")
    make_identity(nc, identity)
    eps_t = ctile([128, 1], FP32, "eps_t")
    nc.vector.memset(eps_t, EPS)
    magic_t = ctile([128, 1], I32, "magic_t")
    nc.vector.memset(magic_t, RSQRT_MAGIC)

    # raw weights, contraction feature on partitions
    w1r = ctile([128, NPC, KC, NF], FP32, "w1r")     # [p, n, k, f]
    for n in range(NPC):
        nc.sync.dma_start(
            out=w1r[:, n],
            in_=bass.AP(tensor=w1_d.tensor, offset=n * NI * NF,
                        ap=[[NF, 128], [128 * NF, KC], [1, NF]]))
    w2r = ctile([128, NPC, NF], FP32, "w2r")         # [p=NF_in, n, f]
    nc.sync.dma_start(
        out=w2r,
        in_=bass.AP(tensor=w2_d.tensor, offset=0,
                    ap=[[NF, 128], [NF * NF, NPC], [1, NF]]))
    w3r = ctile([128, NPC, NO], FP32, "w3r")         # [p=NF, n, o]
    nc.sync.dma_start(
        out=w3r,
        in_=bass.AP(tensor=w3_d.tensor, offset=0,
                    ap=[[NO, 128], [NF * NO, NPC], [1, NO]]))

    # per-neuron vectors, transposed so the feature axis is on partitions
    def load_vecT(dram_ap, width, tag):
        t = ctile([128, width // 128, NPC], FP32, tag)
        for k in range(width // 128):
            nc.gpsimd.dma_start(
                out=t[:, k, :],
                in_=bass.AP(tensor=dram_ap.tensor, offset=k * 128,
                            ap=[[1, 128], [width, NPC]]))
        return t

    g1t = load_vecT(g1_d, NI, "g1t")       # [128, KC, NPC]
    be1t = load_vecT(be1_d, NI, "be1t")
    b1t = load_vecT(b1_d, NF, "b1t")       # [128, 1, NPC]
    g2t = load_vecT(g2_d, NF, "g2t")
    be2t = load_vecT(be2_d, NF, "be2t")
    b2t = load_vecT(b2_d, NF, "b2t")

    # split-bf16 W3: hi = bf16(w3), lo = bf16(w3 - hi)
    w3hi = ctile([128, NPC, NO], BF16, "w3hi")
    nc.gpsimd.tensor_copy(out=w3hi, in_=w3r)
    w3lo = ctile([128, NPC, NO], BF16, "w3lo")
    nc.vector.tensor_tensor(out=w3lo, in0=w3r, in1=w3hi, op=AL.subtract)

    # bf16 copies of raw weights for the tiny bias-fold matmuls (avoids
    # fp32 matmuls entirely — fp32 PE paths have known HW-hang edge cases)
    w1rb = ctile([128, NPC, KC, NF], BF16, "w1rb")
    nc.gpsimd.tensor_copy(out=w1rb, in_=w1r)
    w2rb = ctile([128, NPC, NF], BF16, "w2rb")
    nc.gpsimd.tensor_copy(out=w2rb, in_=w2r)

    def rsqrt(dst, src, scratch_tag):
        """dst = 1/sqrt(src) via bit trick + 3 Newton steps (gpsimd only).

        src must be > 0. dst/src are [128, 1] fp32 APs (may not alias).
        """
        y = dst
        yi = y.bitcast(I32)
        si = src.bitcast(I32)
        t = vec.tile([128, 1], I32, tag=scratch_tag, name=scratch_tag)
        tf = t.bitcast(FP32)
        # y = bitcast(magic - (bitcast(src) >> 1))
        nc.vector.tensor_scalar(out=t, in0=si, scalar1=1, scalar2=None,
                                op0=AL.arith_shift_right)
        nc.vector.tensor_tensor(out=yi, in0=magic_t, in1=t, op=AL.subtract)
        for _ in range(3):
            # y *= 1.5 - 0.5 * src * y * y
            nc.vector.tensor_tensor(out=tf, in0=y, in1=y, op=AL.mult)
            nc.vector.tensor_tensor(out=tf, in0=tf, in1=src, op=AL.mult)
            nc.vector.tensor_scalar(out=tf, in0=tf, scalar1=-0.5, scalar2=1.5,
                                    op0=AL.mult, op1=AL.add)
            nc.vector.tensor_tensor(out=y, in0=y, in1=tf, op=AL.mult)

    # ---------------- x load + transpose + BN1 stats ----------------
    xT = ctile([128, KC, B], BF16, "xT")             # feature-major x
    stats_x = ctile([128, KC, 16, 6], FP32, "stats_x")

    for g in range(4):                               # 4 groups x 16 batch tiles
        pms = [mm_ps.tile([128, CHUNK], FP32, tag="mm", name=f"tp{g}_{k}")
               for k in range(KC)]
        for j in range(16):
            xc = xload.tile([128, NI], FP32, tag="xc", name=f"xc{g}_{j}")
            bt = g * 16 + j
            nc.sync.dma_start(out=xc, in_=x_d[bt * 128:(bt + 1) * 128, :])
            for k in range(KC):
                nc.tensor.transpose(pms[k][:, j * 128:(j + 1) * 128],
                                    xc[:, k * 128:(k + 1) * 128], identity)
        for k in range(KC):
            # evict fp32 psum -> bf16 xT (Copy lives in the tanh table set)
            nc.scalar.activation(out=xT[:, k, g * CHUNK:(g + 1) * CHUNK],
                                 in_=pms[k], func=AF.Copy)
            # BN1 stats on fp32 transposed chunks straight from psum
            for q in range(4):
                nc.vector.bn_stats(out=stats_x[:, k, g * 4 + q, :],
                                   in_=pms[k][:, q * 512:(q + 1) * 512])

    mv1 = ctile([128, KC, 2], FP32, "mv1")
    rs1 = ctile([128, KC], FP32, "rs1")              # 1/sqrt(var1+eps)
    vpe1 = ctile([128, KC], FP32, "vpe1")
    for k in range(KC):
        nc.vector.bn_aggr(out=mv1[:, k, :], in_=stats_x[:, k])
        nc.vector.tensor_scalar(out=vpe1[:, k:k + 1], in0=mv1[:, k, 1:2],
                                scalar1=EPS, scalar2=None, op0=AL.add)
        rsqrt(rs1[:, k:k + 1], vpe1[:, k:k + 1], f"rsq1_{k}")

    # ---------------- per-neuron pipeline ----------------
    h2s = ctile([128, B], BF16, "h2s")               # h2 (single buffer)

    def prep1(n):
        """Folded lin1 weights/bias for neuron n -> (w1e, b1e)."""
        rg = vec.tile([128, KC], FP32, tag="rg", name=f"rg{n}")
        rgn = vec.tile([128, KC], FP32, tag="rgn", name=f"rgn{n}")
        v1 = vec.tile([128, KC], BF16, tag="v1", name=f"v1{n}")
        w1e = wef.tile([128, KC, NF], BF16, tag="w1e", name=f"w1e{n}")
        for k in range(KC):
            nc.vector.tensor_tensor(out=rg[:, k:k + 1], in0=rs1[:, k:k + 1],
                                    in1=g1t[:, k, n:n + 1], op=AL.mult)
            nc.vector.tensor_scalar(out=w1e[:, k], in0=w1r[:, n, k],
                                    scalar1=rg[:, k:k + 1], scalar2=None,
                                    op0=AL.mult)
            nc.vector.tensor_scalar(out=rgn[:, k:k + 1], in0=rg[:, k:k + 1],
                                    scalar1=-1.0, scalar2=None, op0=AL.mult)
            # v1 = beta1 - mu1*rg  ==  (mu1 * (-rg)) + beta1
            nc.vector.scalar_tensor_tensor(
                out=v1[:, k:k + 1], in0=mv1[:, k, 0:1],
                scalar=rgn[:, k:k + 1], in1=be1t[:, k, n:n + 1],
                op0=AL.mult, op1=AL.add)
        bp = mm_ps.tile([128, 1], FP32, tag="mm", name=f"bp1_{n}")
        for k in range(KC):
            nc.tensor.matmul(bp, lhsT=w1rb[:, n, k], rhs=v1[:, k:k + 1],
                             start=(k == 0), stop=(k == KC - 1))
        b1e = vec.tile([128, 1], FP32, tag="b1e", name=f"b1e{n}")
        nc.vector.tensor_tensor(out=b1e, in0=bp, in1=b1t[:, 0, n:n + 1],
                                op=AL.add)
        return w1e, b1e

    def lin1(n, w1e, b1e, h1):
        for c in range(NCH):
            pm = mm_ps.tile([128, CHUNK], FP32, tag="mm", name=f"l1_{n}_{c}")
            for j in range(CHUNK // MMN):
                col = c * CHUNK + j * MMN
                for k in range(KC):
                    nc.tensor.matmul(pm[:, j * MMN:(j + 1) * MMN],
                                     lhsT=w1e[:, k],
                                     rhs=xT[:, k, col:col + MMN],
                                     start=(k == 0), stop=(k == KC - 1))
            nc.scalar.activation(out=h1[:, c * CHUNK:(c + 1) * CHUNK], in_=pm,
                                 func=AF.Tanh, bias=b1e, scale=1.0)

    def prep2(n, h1):
        """BN2 stats (bn_stats over h1) + folded lin2 weights/bias."""
        st2 = vec.tile([128, 16, 6], FP32, tag="st2", name=f"st2_{n}")
        for q in range(16):
            nc.vector.bn_stats(out=st2[:, q, :],
                               in_=h1[:, q * 512:(q + 1) * 512])
        mv2 = vec.tile([128, 2], FP32, tag="mv2", name=f"mv2_{n}")
        nc.vector.bn_aggr(out=mv2, in_=st2)
        mu2 = mv2[:, 0:1]
        vpe = vec.tile([128, 1], FP32, tag="vpe", name=f"vpe_{n}")
        nc.vector.tensor_scalar(out=vpe, in0=mv2[:, 1:2], scalar1=EPS,
                                scalar2=None, op0=AL.add)
        rs2 = vec.tile([128, 1], FP32, tag="rs2", name=f"rs2_{n}")
        rsqrt(rs2, vpe, "rsq2")
        s2 = vec.tile([128, 1], FP32, tag="s2", name=f"s2_{n}")
        nc.vector.tensor_tensor(out=s2, in0=rs2, in1=g2t[:, 0, n:n + 1],
                                op=AL.mult)
        w2e = wef.tile([128, NF], BF16, tag="w2e", name=f"w2e{n}")
        nc.vector.tensor_scalar(out=w2e, in0=w2r[:, n], scalar1=s2,
                                scalar2=None, op0=AL.mult)
        s2n = vec.tile([128, 1], FP32, tag="s2n", name=f"s2n_{n}")
        nc.vector.tensor_scalar(out=s2n, in0=s2, scalar1=-1.0, scalar2=None,
                                op0=AL.mult)
        # t2 = beta2 - mu2*s2
        t2 = vec.tile([128, 1], BF16, tag="t2", name=f"t2_{n}")
        nc.vector.scalar_tensor_tensor(out=t2, in0=mu2, scalar=s2n,
                                       in1=be2t[:, 0, n:n + 1],
                                       op0=AL.mult, op1=AL.add)
        bp = mm_ps.tile([128, 1], FP32, tag="mm", name=f"bp2_{n}")
        nc.tensor.matmul(bp, lhsT=w2rb[:, n], rhs=t2, start=True, stop=True)
        b2e = vec.tile([128, 1], FP32, tag="b2e", name=f"b2e{n}")
        nc.vector.tensor_tensor(out=b2e, in0=bp, in1=b2t[:, 0, n:n + 1],
                                op=AL.add)
        return w2e, b2e

    def lin2(n, h1, w2e, b2e):
        for c in range(NCH):
            pm = mm_ps.tile([128, CHUNK], FP32, tag="mm", name=f"l2_{n}_{c}")
            for j in range(CHUNK // MMN):
                col = c * CHUNK + j * MMN
                nc.tensor.matmul(pm[:, j * MMN:(j + 1) * MMN], lhsT=w2e,
                                 rhs=h1[:, col:col + MMN],
                                 start=True, stop=True)
            nc.scalar.activation(out=h2s[:, c * CHUNK:(c + 1) * CHUNK],
                                 in_=pm, func=AF.Tanh, bias=b2e, scale=1.0)

    def lin3(n, stg):
        """h2 @ (W3hi + W3lo) -> psum; evict (+b3) into pair staging."""
        j = n % 2
        for g in range(2):
            po = mm_ps.tile([128, CHUNK], FP32, tag="mm", name=f"l3_{n}_{g}")
            for t in range(32):
                bt = g * 32 + t
                lhsT = h2s[:, bt * 128:(bt + 1) * 128]
                nc.tensor.matmul(po[:, t * NO:(t + 1) * NO], lhsT=lhsT,
                                 rhs=w3hi[:, n], start=True, stop=False)
                nc.tensor.matmul(po[:, t * NO:(t + 1) * NO], lhsT=lhsT,
                                 rhs=w3lo[:, n], start=False, stop=True)
            # staging [p, t, j, o]; (b3 is added host-side — it is a pure
            # output-space bias, so the post-gather add is exact)
            dst = stg[g][:, :, j, :]
            srcv = po.rearrange("p (t o) -> p t o", o=NO)
            nc.vector.tensor_copy(out=dst, in_=srcv)

    def flush(p, stg):
        """DMA one pair's staged outputs; rows of 2*NO*4 = 512 bytes."""
        for g in range(2):
            src = stg[g].rearrange("p t j o -> p t (j o)")
            dst = bass.AP(
                tensor=out_d.tensor,
                offset=(g * 32 * 128) * (NPC * NO) + p * (2 * NO),
                ap=[[NPC * NO, 128], [128 * NPC * NO, 32], [1, 2 * NO]])
            nc.sync.dma_start(out=dst, in_=src)

    # Depth-2 software pipeline: lin1(n+1) is issued BEFORE lin2(n) so the
    # in-order PE stream has a full lin1 sweep in flight while neuron n's
    # BN2 stats resolve on DVE/GPSIMD — no PE bubble, keeps HAM warm.
    def issue_lin1(n):
        w1e, b1e = prep1(n)
        h1 = hpool.tile([128, B], BF16, tag="h1", name=f"h1_{n}")
        lin1(n, w1e, b1e, h1)
        return h1

    pipe = [issue_lin1(0), issue_lin1(1)]
    stg = None
    for n in range(NPC):
        h1 = pipe[n]
        w2e, b2e = prep2(n, h1)
        lin2(n, h1, w2e, b2e)
        if n + 2 < NPC:
            pipe.append(issue_lin1(n + 2))
        if n % 2 == 0:
            stg = [stg_pool.tile([128, 32, 2, NO], FP32, tag=f"stg{g}",
                                 name=f"stg{g}_{n}") for g in range(2)]
        lin3(n, stg)
        if n % 2 == 1:
            flush(n // 2, stg)


_CACHE = {}


def kernel(**inputs):
    inp = {k: np.asarray(v) for k, v in inputs.items()}
    if "nc" not in _CACHE:
        _CACHE["nc"] = build_program()
    nc = _CACHE["nc"]

    in_maps = []
    for c in range(NCORES):
        sl = slice(c * NPC, (c + 1) * NPC)
        m = {"x": np.ascontiguousarray(inp["x"], dtype=np.float32)}
        for name, key in (("W1s", "W1"), ("b1s", "b1"), ("W2s", "W2"),
                          ("b2s", "b2"), ("W3s", "W3"), ("b3s", "b3"),
                          ("g1s", "g1"), ("beta1s", "beta1"),
                          ("g2s", "g2"), ("beta2s", "beta2")):
            m[name] = np.ascontiguousarray(inp[key][sl], dtype=np.float32)
        in_maps.append(m)

    res = run_bass_kernel_spmd(nc, in_maps, core_ids=list(range(NCORES)))
    out = np.concatenate([res.results[c]["out"] for c in range(NCORES)],
                         axis=1)
    out += np.asarray(inp["b3"], np.float32).reshape(-1)[None, :]
    return out


if __name__ == "__main__":
    nc = build_program()
    print("program built OK")
